# revision 1
# baseline (speedup 1.0000x reference)
import numpy as np
from contextlib import ExitStack

# GCN: 3 message-passing layers + global mean pool + linear head + log_softmax.
# Algebraic split per layer (m = concat([x[src], ea]); agg = segsum(m, dst)):
#   agg @ W = (A @ x) @ W[:128] + S @ W[128:]
# where A = adjacency (+ self loops) and S = segsum(edge_attr, dst) is layer-
# invariant. Host does the sparse A@x (data-dependent gather/scatter) and the
# tiny S/pool math; the 8 NeuronCores do the dense [N,128]@[128,128]+bias+relu
# update, node-sharded 12544 rows per core.

N = 100000
E = 1600000
NG = 100
ED = 4
D = 128
NCORES = 8
PER = 12544            # 98 chunks of 128 rows per core; 8*PER = 100352 >= N
NPAD = NCORES * PER
CHUNKS = PER // 128

_nc = None


def _build():
    global _nc
    if _nc is not None:
        return _nc
    import concourse.bass as bass
    import concourse.tile as tile
    import concourse.bacc as bacc
    from concourse import mybir

    nc = bacc.Bacc("TRN2", target_bir_lowering=False, debug=False,
                   num_devices=NCORES)
    gt = nc.dram_tensor("gt", [D, PER], mybir.dt.float32, kind="ExternalInput").ap()
    w = nc.dram_tensor("w", [D, D], mybir.dt.float32, kind="ExternalInput").ap()
    # S'^T with a ones row folding in the bias: C = S'.T-chunks @ wb
    st = nc.dram_tensor("st", [ED + 1, PER], mybir.dt.float32, kind="ExternalInput").ap()
    wb = nc.dram_tensor("wb", [ED + 1, D], mybir.dt.float32, kind="ExternalInput").ap()
    out = nc.dram_tensor("out", [PER, D], mybir.dt.float32, kind="ExternalOutput").ap()

    with tile.TileContext(nc) as tc:
        with ExitStack() as ctx:
            wpool = ctx.enter_context(tc.tile_pool(name="wpool", bufs=1))
            inpool = ctx.enter_context(tc.tile_pool(name="inpool", bufs=4))
            psum = ctx.enter_context(
                tc.tile_pool(name="psum", bufs=4, space=bass.MemorySpace.PSUM))
            opool = ctx.enter_context(tc.tile_pool(name="opool", bufs=4))

            wt = wpool.tile([D, D], mybir.dt.float32)
            nc.sync.dma_start(wt[:], w[:])
            wbt = wpool.tile([ED + 1, D], mybir.dt.float32)
            nc.sync.dma_start(wbt[:], wb[:])
            s_t = wpool.tile([ED + 1, PER], mybir.dt.float32)
            nc.sync.dma_start(s_t[:], st[:])
            for i in range(CHUNKS):
                g_t = inpool.tile([D, 128], mybir.dt.float32)
                nc.sync.dma_start(g_t[:], gt[:, bass.ts(i, 128)])
                ps = psum.tile([128, D], mybir.dt.float32)
                nc.tensor.matmul(ps[:], g_t[:], wt[:], start=True, stop=False)
                nc.tensor.matmul(ps[:], s_t[:, bass.ts(i, 128)], wbt[:],
                                 start=False, stop=True)
                s2 = opool.tile([128, D], mybir.dt.float32)
                nc.scalar.activation(s2[:], ps[:],
                                     bass.mybir.ActivationFunctionType.Relu)
                nc.sync.dma_start(out[bass.ts(i, 128), :], s2[:])
    nc.compile()
    _nc = nc
    return nc


def _run_layer(g, st_pad, Wa, Wb_aug):
    from concourse.bass_utils import run_bass_kernel_spmd
    nc = _build()
    gpad = np.zeros((NPAD, D), np.float32)
    gpad[:N] = g
    wa = np.ascontiguousarray(Wa, dtype=np.float32)
    wb = np.ascontiguousarray(Wb_aug, dtype=np.float32)
    in_maps = []
    for c in range(NCORES):
        sl = slice(c * PER, (c + 1) * PER)
        in_maps.append({
            "gt": np.ascontiguousarray(gpad[sl].T),
            "w": wa,
            "st": np.ascontiguousarray(st_pad[:, sl]),
            "wb": wb,
        })
    res = run_bass_kernel_spmd(nc, in_maps, core_ids=list(range(NCORES)))
    outs = res.results
    parts = []
    for c in range(NCORES):
        o = outs[c]
        parts.append(o["out"] if isinstance(o, dict) else o)
    h = np.concatenate(parts, axis=0)
    return h[:N]


def kernel(**inputs):
    import scipy.sparse as sp
    x = np.asarray(inputs["x"], dtype=np.float32)
    ei = np.asarray(inputs["edge_index"]).astype(np.int64)
    ea = np.asarray(inputs["edge_attr"], dtype=np.float32)
    batch = np.asarray(inputs["batch"]).astype(np.int64)

    src, dst = ei[0], ei[1]
    ne = ei.shape[1]
    ones_e = np.ones(ne, dtype=np.float32)
    A = sp.csr_matrix((ones_e, (dst, src)), shape=(N, N))
    sel = sp.csr_matrix((ones_e, (dst, np.arange(ne))), shape=(N, ne))
    S = sel @ ea                               # [N,4]; self-loop attrs are zero

    # S augmented with a ones column (folds the bias b into the wb matmul),
    # transposed + padded once; the per-node part is layer-invariant.
    st_pad = np.zeros((ED + 1, NPAD), np.float32)
    st_pad[:ED, :N] = S.T
    st_pad[ED, :N] = 1.0

    h = x
    for Wn, bn in (("W0", "b0"), ("W1", "b1"), ("W2", "b2")):
        W = np.asarray(inputs[Wn], dtype=np.float32)
        b = np.asarray(inputs[bn], dtype=np.float32)
        g = A @ h + h                          # adjacency + self loops
        wb_aug = np.concatenate([W[D:], b[None, :]], axis=0)   # [5,128]
        h = _run_layer(g, st_pad, W[:D], wb_aug)

    pool = sp.csr_matrix(
        (np.ones(N, np.float32), (batch, np.arange(N))), shape=(NG, N))
    counts = np.bincount(batch, minlength=NG).astype(np.float32)
    pooled = (pool @ h) / np.maximum(counts, 1.0)[:, None]
    logits = pooled @ np.asarray(inputs["Wout"], np.float32) \
        + np.asarray(inputs["bout"], np.float32)
    mx = logits.max(axis=1, keepdims=True)
    lse = np.log(np.exp(logits - mx).sum(axis=1, keepdims=True)) + mx
    return (logits - lse).astype(np.float32)



# revision 2
# speedup vs baseline: 2.3016x; 2.3016x over previous
import numpy as np
from contextlib import ExitStack

# GCN: 3 message-passing layers + global mean pool + linear head + log_softmax.
# Algebraic split per layer (m = concat([x[src], ea]); agg = segsum(m, dst)):
#   agg @ W = (A @ x) @ W[:128] + S @ W[128:]
# where A = adjacency (+ self loops) and S = segsum(edge_attr, dst) is layer-
# invariant. Host does the sparse A@x (data-dependent gather/scatter) and the
# tiny S/pool math; the 8 NeuronCores do the dense [N,128]@[128,128]+bias+relu
# update, node-sharded 12544 rows per core. All device I/O is fp16 to halve
# transfer bytes over the slow tunnel; one jitted executable is built once and
# reused for all three layers (outputs donation-chained back in as the
# pre-zeroed output buffers).

N = 100000
E = 1600000
NG = 100
ED = 4
D = 128
NCORES = 8
PER = 12544            # 98 chunks of 128 rows per core; 8*PER = 100352 >= N
NPAD = NCORES * PER
CHUNKS = PER // 128

_STATE = {}


def _build_nc():
    import concourse.bass as bass
    import concourse.tile as tile
    import concourse.bacc as bacc
    from concourse import mybir

    nc = bacc.Bacc("TRN2", target_bir_lowering=False, debug=False,
                   num_devices=NCORES)
    f16 = mybir.dt.float16
    gt = nc.dram_tensor("gt", [D, PER], f16, kind="ExternalInput").ap()
    w = nc.dram_tensor("w", [D, D], f16, kind="ExternalInput").ap()
    # S'^T with a ones row folding in the bias: C = S'.T-chunks @ wb
    st = nc.dram_tensor("st", [ED + 1, PER], f16, kind="ExternalInput").ap()
    wb = nc.dram_tensor("wb", [ED + 1, D], f16, kind="ExternalInput").ap()
    out = nc.dram_tensor("out", [PER, D], f16, kind="ExternalOutput").ap()

    with tile.TileContext(nc) as tc:
        with ExitStack() as ctx:
            wpool = ctx.enter_context(tc.tile_pool(name="wpool", bufs=1))
            inpool = ctx.enter_context(tc.tile_pool(name="inpool", bufs=4))
            psum = ctx.enter_context(
                tc.tile_pool(name="psum", bufs=4, space=bass.MemorySpace.PSUM))
            opool = ctx.enter_context(tc.tile_pool(name="opool", bufs=4))

            wt = wpool.tile([D, D], f16)
            nc.sync.dma_start(wt[:], w[:])
            wbt = wpool.tile([ED + 1, D], f16)
            nc.sync.dma_start(wbt[:], wb[:])
            s_t = wpool.tile([ED + 1, PER], f16)
            nc.sync.dma_start(s_t[:], st[:])
            for i in range(CHUNKS):
                g_t = inpool.tile([D, 128], f16)
                nc.sync.dma_start(g_t[:], gt[:, bass.ts(i, 128)])
                ps = psum.tile([128, D], mybir.dt.float32)
                nc.tensor.matmul(ps[:], g_t[:], wt[:], start=True, stop=False)
                nc.tensor.matmul(ps[:], s_t[:, bass.ts(i, 128)], wbt[:],
                                 start=False, stop=True)
                s2 = opool.tile([128, D], f16)
                nc.scalar.activation(s2[:], ps[:],
                                     bass.mybir.ActivationFunctionType.Relu)
                nc.sync.dma_start(out[bass.ts(i, 128), :], s2[:])
    nc.compile()
    return nc


def _ensure_ready():
    if "fn" in _STATE:
        return
    import jax
    import numpy as _np
    from jax.sharding import Mesh, PartitionSpec
    from jax.experimental.shard_map import shard_map
    from concourse import bass2jax, mybir

    bass2jax.install_neuronx_cc_hook()
    nc = _build_nc()
    assert nc.dbg_addr is None or not nc.dbg_callbacks

    partition_name = (nc.partition_id_tensor.name
                      if nc.partition_id_tensor else None)
    in_names, out_names, out_avals = [], [], []
    for alloc in nc.m.functions[0].allocations:
        if not isinstance(alloc, mybir.MemoryLocationSet):
            continue
        name = alloc.memorylocations[0].name
        if alloc.kind == "ExternalInput":
            if name != partition_name:
                in_names.append(name)
        elif alloc.kind == "ExternalOutput":
            out_names.append(name)
            out_avals.append(jax.core.ShapedArray(
                tuple(alloc.tensor_shape), mybir.dt.np(alloc.dtype)))
    n_params = len(in_names)
    all_in = list(in_names) + list(out_names)
    if partition_name is not None:
        all_in.append(partition_name)

    def _body(*args):
        operands = list(args)
        if partition_name is not None:
            operands.append(bass2jax.partition_id_tensor())
        outs = bass2jax._bass_exec_p.bind(
            *operands,
            out_avals=tuple(out_avals),
            in_names=tuple(all_in),
            out_names=tuple(out_names),
            lowering_input_output_aliases=(),
            sim_require_finite=True,
            sim_require_nnan=True,
            nc=nc,
        )
        return tuple(outs)

    mesh = Mesh(_np.asarray(jax.devices()[:NCORES]), ("core",))
    nin = n_params + len(out_names)
    fn = jax.jit(
        shard_map(_body, mesh=mesh,
                  in_specs=(PartitionSpec("core"),) * nin,
                  out_specs=(PartitionSpec("core"),) * len(out_names),
                  check_rep=False),
        donate_argnums=tuple(range(n_params, nin)),
    )
    _STATE["fn"] = fn
    _STATE["in_names"] = in_names

    # Warm the whole path (XLA + NEFF compile + device load) with dummy data.
    f16 = np.float16
    dummy = {
        "gt": np.zeros((NCORES * D, PER), f16),
        "w": np.zeros((NCORES * D, D), f16),
        "st": np.zeros((NCORES * (ED + 1), PER), f16),
        "wb": np.zeros((NCORES * (ED + 1), D), f16),
    }
    args = [dummy[n] for n in in_names] + [np.zeros((NPAD, D), f16)]
    (out,) = fn(*args)
    out.block_until_ready()
    _STATE["spare_out"] = out   # donated as the first real call's out buffer


def _run_layer(gt_g, w_g, st_g, wb_g):
    fn = _STATE["fn"]
    glob = {"gt": gt_g, "w": w_g, "st": st_g, "wb": wb_g}
    args = [glob[n] for n in _STATE["in_names"]] + [_STATE["spare_out"]]
    (out,) = fn(*args)
    h = np.asarray(out)          # [NPAD, D] f16, global node order
    _STATE["spare_out"] = out    # chain-donate into the next call
    return h


def kernel(**inputs):
    import scipy.sparse as sp
    _ensure_ready()

    x = np.asarray(inputs["x"], dtype=np.float32)
    ei = np.asarray(inputs["edge_index"]).astype(np.int64)
    ea = np.asarray(inputs["edge_attr"], dtype=np.float32)
    batch = np.asarray(inputs["batch"]).astype(np.int64)

    src, dst = ei[0], ei[1]
    ne = ei.shape[1]
    ones_e = np.ones(ne, dtype=np.float32)
    A = sp.csr_matrix((ones_e, (dst, src)), shape=(N, N))
    sel = sp.csr_matrix((ones_e, (dst, np.arange(ne))), shape=(N, ne))
    S = sel @ ea                               # [N,4]; self-loop attrs are zero

    # S augmented with a ones column (folds the bias b into the wb matmul),
    # transposed + padded once; the per-node part is layer-invariant.
    st_pad = np.zeros((ED + 1, NPAD), np.float32)
    st_pad[:ED, :N] = S.T
    st_pad[ED, :N] = 1.0
    st_g = np.ascontiguousarray(
        st_pad.reshape(ED + 1, NCORES, PER).transpose(1, 0, 2)
    ).reshape(NCORES * (ED + 1), PER).astype(np.float16)

    h = x
    for Wn, bn in (("W0", "b0"), ("W1", "b1"), ("W2", "b2")):
        W = np.asarray(inputs[Wn], dtype=np.float32)
        b = np.asarray(inputs[bn], dtype=np.float32)
        g = A @ h + h                          # adjacency + self loops
        gpad = np.zeros((NPAD, D), np.float32)
        gpad[:N] = g
        # per-core transposed layout: [8, D, PER] flattened on axis 0
        gt_g = np.ascontiguousarray(
            gpad.reshape(NCORES, PER, D).transpose(0, 2, 1)
        ).reshape(NCORES * D, PER).astype(np.float16)
        w_g = np.tile(W[:D].astype(np.float16), (NCORES, 1))
        wb_aug = np.concatenate([W[D:], b[None, :]], axis=0)   # [5,128]
        wb_g = np.tile(wb_aug.astype(np.float16), (NCORES, 1))
        hpad = _run_layer(gt_g, w_g, st_g, wb_g)
        h = hpad[:N].astype(np.float32)

    pool = sp.csr_matrix(
        (np.ones(N, np.float32), (batch, np.arange(N))), shape=(NG, N))
    counts = np.bincount(batch, minlength=NG).astype(np.float32)
    pooled = (pool @ h) / np.maximum(counts, 1.0)[:, None]
    logits = pooled @ np.asarray(inputs["Wout"], np.float32) \
        + np.asarray(inputs["bout"], np.float32)
    mx = logits.max(axis=1, keepdims=True)
    lse = np.log(np.exp(logits - mx).sum(axis=1, keepdims=True)) + mx
    return (logits - lse).astype(np.float32)


try:
    _ensure_ready()
except Exception:
    _STATE.clear()


# revision 3
# speedup vs baseline: 10.1928x; 4.4286x over previous
import numpy as np
from contextlib import ExitStack

# GCN: 3 message-passing layers + global mean pool + linear head + log_softmax,
# run end-to-end on 8 NeuronCores in ONE device invocation.
#
# Sharding: core c owns PER=12544 consecutive nodes (98 windows of 128).
# Host buckets the edges by destination window (dst>>7), padding each window
# to K=19*128 slots (max real count is ~2200); padding edges point at the
# all-zero row NPAD-1. Per layer, per window, a core indirect-DMA-gathers
# h[src] for the window's edges (128 rows/instr), segment-sums them with a
# one-hot matmul (dst_local == iota) accumulating the transposed aggregate in
# PSUM, adds the self-loop via an identity matmul, then applies the dense
# update (g^T @ W + st @ wb, relu). AllGather replicates h between layers.
# The last layer accumulates per-graph pooled partials (batch one-hot
# matmul); only those [100,128] partials are downloaded, and the tiny head
# (mean, Wout, log_softmax) runs on host. Device I/O is fp16 (fp32 PSUM).
#
# The edge split per layer uses agg @ W = (A@h + h) @ W[:128] + S @ W[128:]
# with S = segsum(edge_attr by dst) layer-invariant, so edge attributes never
# touch the device per-edge.

N = 100000
E = 1600000
NG = 100
ED = 4
D = 128
NC = 8
PER = 12544
NW = 98          # windows of 128 nodes per core
KT = 19          # 128-edge tiles per window (2432 slots >= max ~2200)
NPAD = NC * PER
NT = NW * KT
K = KT * 128

_STATE = {}


def _build_nc():
    import concourse.bass as bass
    import concourse.tile as tile
    import concourse.bacc as bacc
    from concourse import mybir

    nc = bacc.Bacc("TRN2", target_bir_lowering=False, debug=False,
                   num_devices=NC)
    f16 = mybir.dt.float16
    f32 = mybir.dt.float32
    i32 = mybir.dt.int32
    Relu = mybir.ActivationFunctionType.Relu
    iseq = mybir.AluOpType.is_equal

    x_l = nc.dram_tensor("x", [PER, D], f16, kind="ExternalInput").ap()
    idx_d = nc.dram_tensor("idx", [128, NT], i32, kind="ExternalInput").ap()
    dstl_d = nc.dram_tensor("dstl", [128, NT], f16, kind="ExternalInput").ap()
    stv_d = nc.dram_tensor("stv", [5, PER], f16, kind="ExternalInput").ap()
    batch_d = nc.dram_tensor("batchv", [128, NW], f16,
                             kind="ExternalInput").ap()
    iota_d = nc.dram_tensor("iota", [128, 128], f16, kind="ExternalInput").ap()
    ident_d = nc.dram_tensor("ident", [128, 128], f16,
                             kind="ExternalInput").ap()
    ws_d, wbs_d = [], []
    for li in range(3):
        ws_d.append(nc.dram_tensor(f"w{li}", [D, D], f16,
                                   kind="ExternalInput").ap())
        wbs_d.append(nc.dram_tensor(f"wb{li}", [5, D], f16,
                                    kind="ExternalInput").ap())
    pooled = nc.dram_tensor("pooled", [NG, D], f32, kind="ExternalOutput").ap()

    h_tab = [nc.dram_tensor(f"htab{i}", [NPAD, D], f16, kind="Internal",
                            addr_space="Shared").ap() for i in range(3)]
    h_loc = [nc.dram_tensor(f"hloc{i}", [PER, D], f16, kind="Internal").ap()
             for i in range(3)]
    groups = [list(range(NC))]

    with tile.TileContext(nc) as tc:
        with ExitStack() as ctx:
            cpool = ctx.enter_context(tc.tile_pool(name="cpool", bufs=1))
            mpool = ctx.enter_context(tc.tile_pool(name="mpool", bufs=4))
            opool = ctx.enter_context(tc.tile_pool(name="opool", bufs=4))
            pspool = ctx.enter_context(
                tc.tile_pool(name="pspool", bufs=2, space="PSUM"))
            ps2pool = ctx.enter_context(
                tc.tile_pool(name="ps2pool", bufs=2, space="PSUM"))
            ps3pool = ctx.enter_context(
                tc.tile_pool(name="ps3pool", bufs=2, space="PSUM"))

            idx_s = cpool.tile([128, NT], i32)
            nc.sync.dma_start(idx_s[:], idx_d[:])
            dstl_s = cpool.tile([128, NT], f16)
            nc.sync.dma_start(dstl_s[:], dstl_d[:])
            stv_s = cpool.tile([5, PER], f16)
            nc.sync.dma_start(stv_s[:], stv_d[:])
            batch_s = cpool.tile([128, NW], f16)
            nc.sync.dma_start(batch_s[:], batch_d[:])
            iota_s = cpool.tile([128, 128], f16)
            nc.sync.dma_start(iota_s[:], iota_d[:])
            ident_s = cpool.tile([128, 128], f16)
            nc.sync.dma_start(ident_s[:], ident_d[:])
            w_s, wb_s = [], []
            for li in range(3):
                wt = cpool.tile([D, D], f16)
                nc.sync.dma_start(wt[:], ws_d[li][:])
                w_s.append(wt)
                wbt = cpool.tile([5, D], f16)
                nc.sync.dma_start(wbt[:], wbs_d[li][:])
                wb_s.append(wbt)
            pool_acc = cpool.tile([NG, D], f32)
            nc.vector.memset(pool_acc[:], 0.0)

            nc.gpsimd.dma_start(h_loc[0][:], x_l[:])
            nc.gpsimd.collective_compute(
                "AllGather", mybir.AluOpType.bypass, replica_groups=groups,
                ins=[h_loc[0][:]], outs=[h_tab[0][:]])

            for li in range(3):
                last = li == 2
                with tc.For_i(0, NW) as w:
                    psg = pspool.tile([128, 128], f32, space="PSUM")
                    idx_w = mpool.tile([128, KT], i32)
                    nc.vector.tensor_copy(idx_w[:], idx_s[:, bass.ts(w, KT)])
                    dstl_w = mpool.tile([128, KT], f16)
                    nc.vector.tensor_copy(dstl_w[:], dstl_s[:, bass.ts(w, KT)])
                    for t in range(KT):
                        msg = mpool.tile([128, D], f16)
                        nc.gpsimd.indirect_dma_start(
                            out=msg[:],
                            out_offset=None,
                            in_=h_tab[li][:],
                            in_offset=bass.IndirectOffsetOnAxis(
                                ap=idx_w[:, t:t + 1], axis=0),
                        )
                        oneh = mpool.tile([128, 128], f16)
                        nc.vector.tensor_tensor(
                            out=oneh[:],
                            in0=dstl_w[:, t:t + 1].to_broadcast([128, 128]),
                            in1=iota_s[:],
                            op=iseq)
                        nc.tensor.matmul(psg[:], msg[:], oneh[:],
                                         start=(t == 0), stop=False)
                    hw = mpool.tile([128, D], f16)
                    nc.sync.dma_start(hw[:], h_loc[li][bass.ts(w, 128), :])
                    nc.tensor.matmul(psg[:], hw[:], ident_s[:],
                                     start=False, stop=True)
                    gT = opool.tile([128, 128], f16)
                    nc.vector.tensor_copy(gT[:], psg[:])
                    ps2 = ps2pool.tile([128, D], f32, space="PSUM")
                    nc.tensor.matmul(ps2[:], gT[:], w_s[li][:],
                                     start=True, stop=False)
                    stw = mpool.tile([5, 128], f16)
                    nc.vector.tensor_copy(stw[:], stv_s[:, bass.ts(w, 128)])
                    nc.tensor.matmul(ps2[:], stw[:], wb_s[li][:],
                                     start=False, stop=True)
                    hn = opool.tile([128, D], f16)
                    nc.scalar.activation(hn[:], ps2[:], Relu)
                    if not last:
                        nc.sync.dma_start(h_loc[li + 1][bass.ts(w, 128), :],
                                          hn[:])
                    else:
                        onehB = opool.tile([128, NG], f16)
                        nc.vector.tensor_tensor(
                            out=onehB[:],
                            in0=batch_s[:, bass.ds(w, 1)].to_broadcast(
                                [128, NG]),
                            in1=iota_s[:, :NG],
                            op=iseq)
                        ps3 = ps3pool.tile([NG, D], f32, space="PSUM")
                        nc.tensor.matmul(ps3[:], onehB[:], hn[:],
                                         start=True, stop=True)
                        nc.vector.tensor_add(pool_acc[:], pool_acc[:], ps3[:])
                if not last:
                    nc.gpsimd.collective_compute(
                        "AllGather", mybir.AluOpType.bypass,
                        replica_groups=groups,
                        ins=[h_loc[li + 1][:]], outs=[h_tab[li + 1][:]])
            nc.sync.dma_start(pooled[:], pool_acc[:])
    nc.compile()
    return nc


def _ensure_ready():
    if "fn" in _STATE:
        return
    import jax
    from jax.sharding import Mesh, PartitionSpec
    from jax.experimental.shard_map import shard_map
    from concourse import bass2jax, mybir

    bass2jax.install_neuronx_cc_hook()
    nc = _build_nc()

    partition_name = (nc.partition_id_tensor.name
                      if nc.partition_id_tensor else None)
    in_names, out_names, out_avals = [], [], []
    for alloc in nc.m.functions[0].allocations:
        if not isinstance(alloc, mybir.MemoryLocationSet):
            continue
        name = alloc.memorylocations[0].name
        if alloc.kind == "ExternalInput":
            if name != partition_name:
                in_names.append(name)
        elif alloc.kind == "ExternalOutput":
            out_names.append(name)
            out_avals.append(jax.core.ShapedArray(
                tuple(alloc.tensor_shape), mybir.dt.np(alloc.dtype)))
    n_params = len(in_names)
    all_in = list(in_names) + list(out_names)
    if partition_name is not None:
        all_in.append(partition_name)

    def _body(*args):
        operands = list(args)
        if partition_name is not None:
            operands.append(bass2jax.partition_id_tensor())
        outs = bass2jax._bass_exec_p.bind(
            *operands,
            out_avals=tuple(out_avals),
            in_names=tuple(all_in),
            out_names=tuple(out_names),
            lowering_input_output_aliases=(),
            sim_require_finite=True,
            sim_require_nnan=True,
            nc=nc,
        )
        return tuple(outs)

    mesh = Mesh(np.asarray(jax.devices()[:NC]), ("core",))
    nin = n_params + len(out_names)
    fn = jax.jit(
        shard_map(_body, mesh=mesh,
                  in_specs=(PartitionSpec("core"),) * nin,
                  out_specs=(PartitionSpec("core"),) * len(out_names),
                  check_rep=False),
        donate_argnums=tuple(range(n_params, nin)),
    )
    _STATE["fn"] = fn
    _STATE["in_names"] = in_names

    # Warm the whole path (XLA + NEFF compile + device load) with dummy data.
    f16 = np.float16
    dummy = _dummy_inputs()
    args = [dummy[n] for n in in_names] + [np.zeros((NC * NG, D), np.float32)]
    (out,) = fn(*args)
    out.block_until_ready()


def _dummy_inputs():
    f16 = np.float16
    return {
        "x": np.zeros((NPAD, D), f16),
        "idx": np.zeros((NC * 128, NT), np.int32),
        "dstl": np.zeros((NC * 128, NT), f16),
        "stv": np.zeros((NC * 5, PER), f16),
        "batchv": np.zeros((NC * 128, NW), f16),
        "iota": np.zeros((NC * 128, 128), f16),
        "ident": np.zeros((NC * 128, 128), f16),
        **{f"w{li}": np.zeros((NC * D, D), f16) for li in range(3)},
        **{f"wb{li}": np.zeros((NC * 5, D), f16) for li in range(3)},
    }


def kernel(**inputs):
    _ensure_ready()
    f16 = np.float16

    x = np.asarray(inputs["x"], dtype=np.float32)
    ei = np.asarray(inputs["edge_index"]).astype(np.int64)
    ea = np.asarray(inputs["edge_attr"], dtype=np.float32)
    batch = np.asarray(inputs["batch"]).astype(np.int64)
    src, dst = ei[0], ei[1]

    # bucket edges by destination window, pad windows to K slots
    order = np.argsort(dst, kind="stable")
    dst_s = dst[order]
    src_s = src[order]
    win = dst_s >> 7
    counts = np.bincount(win, minlength=NC * NW)
    assert counts.max() <= K, f"window overflow: {counts.max()} > {K}"
    starts = np.zeros(NC * NW + 1, np.int64)
    np.cumsum(counts, out=starts[1:])
    slot = np.arange(E) - starts[win]
    src_pad = np.full((NC * NW, K), NPAD - 1, np.int32)
    dstl_pad = np.zeros((NC * NW, K), f16)
    src_pad[win, slot] = src_s.astype(np.int32)
    dstl_pad[win, slot] = (dst_s & 127).astype(f16)
    idx_g = np.ascontiguousarray(
        src_pad.reshape(NC, NW, KT, 128).transpose(0, 3, 1, 2)
    ).reshape(NC * 128, NT)
    dstl_g = np.ascontiguousarray(
        dstl_pad.reshape(NC, NW, KT, 128).transpose(0, 3, 1, 2)
    ).reshape(NC * 128, NT)

    # S = segsum(edge_attr by dst) with a ones row folding in the bias
    stv = np.zeros((5, NPAD), f16)
    for k in range(ED):
        stv[k] = np.bincount(dst, weights=ea[:, k].astype(np.float64),
                             minlength=NPAD).astype(f16)
    stv[4, :N] = 1.0
    stv_g = np.ascontiguousarray(
        stv.reshape(5, NC, PER).transpose(1, 0, 2)).reshape(NC * 5, PER)

    bpad = np.full(NPAD, 127, np.int64)
    bpad[:N] = batch
    batch_g = np.ascontiguousarray(
        bpad.reshape(NC, NW, 128).transpose(0, 2, 1)
    ).astype(f16).reshape(NC * 128, NW)

    xpad = np.zeros((NPAD, D), f16)
    xpad[:N] = x.astype(f16)

    iota_g = np.tile(np.arange(128, dtype=f16), (NC * 128, 1))
    ident_g = np.tile(np.eye(128, dtype=f16), (NC, 1))

    glob = {
        "x": xpad, "idx": idx_g, "dstl": dstl_g, "stv": stv_g,
        "batchv": batch_g, "iota": iota_g, "ident": ident_g,
    }
    for li, (Wn, bn) in enumerate((("W0", "b0"), ("W1", "b1"), ("W2", "b2"))):
        W = np.asarray(inputs[Wn], dtype=np.float32)
        b = np.asarray(inputs[bn], dtype=np.float32)
        glob[f"w{li}"] = np.tile(W[:D].astype(f16), (NC, 1))
        glob[f"wb{li}"] = np.tile(
            np.concatenate([W[D:], b[None, :]], axis=0).astype(f16), (NC, 1))

    fn = _STATE["fn"]
    args = [glob[n] for n in _STATE["in_names"]] \
        + [np.zeros((NC * NG, D), np.float32)]
    (out,) = fn(*args)
    pooled_parts = np.asarray(out).reshape(NC, NG, D)
    pooled_sum = pooled_parts.sum(axis=0)

    counts_g = np.bincount(batch, minlength=NG).astype(np.float32)
    pooled = pooled_sum / np.maximum(counts_g, 1.0)[:, None]
    logits = pooled @ np.asarray(inputs["Wout"], np.float32) \
        + np.asarray(inputs["bout"], np.float32)
    mx = logits.max(axis=1, keepdims=True)
    lse = np.log(np.exp(logits - mx).sum(axis=1, keepdims=True)) + mx
    return (logits - lse).astype(np.float32)


try:
    _ensure_ready()
except Exception:
    _STATE.clear()


# revision 5
# speedup vs baseline: 12.0396x; 1.1812x over previous
import numpy as np
from contextlib import ExitStack

# GCN: 3 message-passing layers + global mean pool + linear head + log_softmax,
# run end-to-end on 8 NeuronCores in ONE device invocation.
#
# Sharding: core c owns PER=12544 consecutive nodes (98 windows of 128).
# Host buckets the edges by destination window (dst>>7), padding each window
# to K=19*128 slots (max real count is ~2200); padding edges point at the
# all-zero row NPAD-1. Per layer, per window, a core indirect-DMA-gathers
# h[src] for the window's edges (128 rows/instr), segment-sums them with a
# one-hot matmul (dst_local == iota) accumulating the transposed aggregate in
# PSUM, adds the self-loop via an identity matmul, then applies the dense
# update (g^T @ W + st @ wb, relu). AllGather replicates h between layers.
# The last layer accumulates per-graph pooled partials (batch one-hot
# matmul); only those [100,128] partials are downloaded, and the tiny head
# (mean, Wout, log_softmax) runs on host. Device I/O is fp16 (fp32 PSUM).
#
# The edge split per layer uses agg @ W = (A@h + h) @ W[:128] + S @ W[128:]
# with S = segsum(edge_attr by dst) layer-invariant, so edge attributes never
# touch the device per-edge.

N = 100000
E = 1600000
NG = 100
ED = 4
D = 128
NC = 8
PER = 12544
NW = 98          # windows of 128 nodes per core
KT = 19          # 128-edge tiles per window (2432 slots >= max ~2200)
NPAD = NC * PER
NT = NW * KT
K = KT * 128

_STATE = {}


def _build_nc():
    import concourse.bass as bass
    import concourse.tile as tile
    import concourse.bacc as bacc
    from concourse import mybir

    nc = bacc.Bacc("TRN2", target_bir_lowering=False, debug=False,
                   num_devices=NC)
    f16 = mybir.dt.float16
    f32 = mybir.dt.float32
    i32 = mybir.dt.int32
    Relu = mybir.ActivationFunctionType.Relu
    iseq = mybir.AluOpType.is_equal

    x_l = nc.dram_tensor("x", [PER, D], f16, kind="ExternalInput").ap()
    idx_d = nc.dram_tensor("idx", [128, NT], i32, kind="ExternalInput").ap()
    dstl_d = nc.dram_tensor("dstl", [128, NT], f16, kind="ExternalInput").ap()
    stv_d = nc.dram_tensor("stv", [5, PER], f16, kind="ExternalInput").ap()
    batch_d = nc.dram_tensor("batchv", [128, NW], f16,
                             kind="ExternalInput").ap()
    iota_d = nc.dram_tensor("iota", [128, 128], f16, kind="ExternalInput").ap()
    ident_d = nc.dram_tensor("ident", [128, 128], f16,
                             kind="ExternalInput").ap()
    ws_d, wbs_d = [], []
    for li in range(3):
        ws_d.append(nc.dram_tensor(f"w{li}", [D, D], f16,
                                   kind="ExternalInput").ap())
        wbs_d.append(nc.dram_tensor(f"wb{li}", [5, D], f16,
                                    kind="ExternalInput").ap())
    pooled = nc.dram_tensor("pooled", [NG, D], f32, kind="ExternalOutput").ap()

    h_tab = [nc.dram_tensor(f"htab{i}", [NPAD, D], f16, kind="Internal",
                            addr_space="Shared").ap() for i in range(3)]
    h_loc = [nc.dram_tensor(f"hloc{i}", [PER, D], f16, kind="Internal").ap()
             for i in range(3)]
    groups = [list(range(NC))]

    with tile.TileContext(nc) as tc:
        with ExitStack() as ctx:
            cpool = ctx.enter_context(tc.tile_pool(name="cpool", bufs=1))
            mpool = ctx.enter_context(tc.tile_pool(name="mpool", bufs=4))
            opool = ctx.enter_context(tc.tile_pool(name="opool", bufs=4))
            pspool = ctx.enter_context(
                tc.tile_pool(name="pspool", bufs=2, space="PSUM"))
            ps2pool = ctx.enter_context(
                tc.tile_pool(name="ps2pool", bufs=2, space="PSUM"))
            ps3pool = ctx.enter_context(
                tc.tile_pool(name="ps3pool", bufs=2, space="PSUM"))

            idx_s = cpool.tile([128, NT], i32)
            nc.sync.dma_start(idx_s[:], idx_d[:])
            dstl_s = cpool.tile([128, NT], f16)
            nc.sync.dma_start(dstl_s[:], dstl_d[:])
            stv_s = cpool.tile([5, PER], f16)
            nc.sync.dma_start(stv_s[:], stv_d[:])
            batch_s = cpool.tile([128, NW], f16)
            nc.sync.dma_start(batch_s[:], batch_d[:])
            iota_s = cpool.tile([128, 128], f16)
            nc.sync.dma_start(iota_s[:], iota_d[:])
            ident_s = cpool.tile([128, 128], f16)
            nc.sync.dma_start(ident_s[:], ident_d[:])
            w_s, wb_s = [], []
            for li in range(3):
                wt = cpool.tile([D, D], f16)
                nc.sync.dma_start(wt[:], ws_d[li][:])
                w_s.append(wt)
                wbt = cpool.tile([5, D], f16)
                nc.sync.dma_start(wbt[:], wbs_d[li][:])
                wb_s.append(wbt)
            pool_acc = cpool.tile([NG, D], f32)
            nc.vector.memset(pool_acc[:], 0.0)

            nc.gpsimd.dma_start(h_loc[0][:], x_l[:])
            nc.gpsimd.collective_compute(
                "AllGather", mybir.AluOpType.bypass, replica_groups=groups,
                ins=[h_loc[0][:]], outs=[h_tab[0][:]])

            for li in range(3):
                last = li == 2
                with tc.For_i(0, NW) as w:
                    psg = pspool.tile([128, 128], f32, space="PSUM")
                    idx_w = mpool.tile([128, KT], i32)
                    nc.vector.tensor_copy(idx_w[:], idx_s[:, bass.ts(w, KT)])
                    dstl_w = mpool.tile([128, KT], f16)
                    nc.vector.tensor_copy(dstl_w[:], dstl_s[:, bass.ts(w, KT)])
                    for t in range(KT):
                        msg = mpool.tile([128, D], f16)
                        nc.gpsimd.indirect_dma_start(
                            out=msg[:],
                            out_offset=None,
                            in_=h_tab[li][:],
                            in_offset=bass.IndirectOffsetOnAxis(
                                ap=idx_w[:, t:t + 1], axis=0),
                        )
                        oneh = mpool.tile([128, 128], f16)
                        nc.vector.tensor_tensor(
                            out=oneh[:],
                            in0=dstl_w[:, t:t + 1].to_broadcast([128, 128]),
                            in1=iota_s[:],
                            op=iseq)
                        nc.tensor.matmul(psg[:], msg[:], oneh[:],
                                         start=(t == 0), stop=False)
                    hw = mpool.tile([128, D], f16)
                    nc.sync.dma_start(hw[:], h_loc[li][bass.ts(w, 128), :])
                    nc.tensor.matmul(psg[:], hw[:], ident_s[:],
                                     start=False, stop=True)
                    gT = opool.tile([128, 128], f16)
                    nc.vector.tensor_copy(gT[:], psg[:])
                    ps2 = ps2pool.tile([128, D], f32, space="PSUM")
                    nc.tensor.matmul(ps2[:], gT[:], w_s[li][:],
                                     start=True, stop=False)
                    stw = mpool.tile([5, 128], f16)
                    nc.vector.tensor_copy(stw[:], stv_s[:, bass.ts(w, 128)])
                    nc.tensor.matmul(ps2[:], stw[:], wb_s[li][:],
                                     start=False, stop=True)
                    hn = opool.tile([128, D], f16)
                    nc.scalar.activation(hn[:], ps2[:], Relu)
                    if not last:
                        nc.sync.dma_start(h_loc[li + 1][bass.ts(w, 128), :],
                                          hn[:])
                    else:
                        onehB = opool.tile([128, NG], f16)
                        nc.vector.tensor_tensor(
                            out=onehB[:],
                            in0=batch_s[:, bass.ds(w, 1)].to_broadcast(
                                [128, NG]),
                            in1=iota_s[:, :NG],
                            op=iseq)
                        ps3 = ps3pool.tile([NG, D], f32, space="PSUM")
                        nc.tensor.matmul(ps3[:], onehB[:], hn[:],
                                         start=True, stop=True)
                        nc.vector.tensor_add(pool_acc[:], pool_acc[:], ps3[:])
                if not last:
                    nc.gpsimd.collective_compute(
                        "AllGather", mybir.AluOpType.bypass,
                        replica_groups=groups,
                        ins=[h_loc[li + 1][:]], outs=[h_tab[li + 1][:]])
            nc.sync.dma_start(pooled[:], pool_acc[:])
    nc.compile()
    return nc


def _ensure_ready():
    if "fn" in _STATE:
        return
    import jax
    from jax.sharding import Mesh, PartitionSpec
    from jax.experimental.shard_map import shard_map
    from concourse import bass2jax, mybir

    bass2jax.install_neuronx_cc_hook()
    nc = _build_nc()

    partition_name = (nc.partition_id_tensor.name
                      if nc.partition_id_tensor else None)
    in_names, out_names, out_avals = [], [], []
    for alloc in nc.m.functions[0].allocations:
        if not isinstance(alloc, mybir.MemoryLocationSet):
            continue
        name = alloc.memorylocations[0].name
        if alloc.kind == "ExternalInput":
            if name != partition_name:
                in_names.append(name)
        elif alloc.kind == "ExternalOutput":
            out_names.append(name)
            out_avals.append(jax.core.ShapedArray(
                tuple(alloc.tensor_shape), mybir.dt.np(alloc.dtype)))
    n_params = len(in_names)
    all_in = list(in_names) + list(out_names)
    if partition_name is not None:
        all_in.append(partition_name)

    def _body(*args):
        operands = list(args)
        if partition_name is not None:
            operands.append(bass2jax.partition_id_tensor())
        outs = bass2jax._bass_exec_p.bind(
            *operands,
            out_avals=tuple(out_avals),
            in_names=tuple(all_in),
            out_names=tuple(out_names),
            lowering_input_output_aliases=(),
            sim_require_finite=True,
            sim_require_nnan=True,
            nc=nc,
        )
        return tuple(outs)

    mesh = Mesh(np.asarray(jax.devices()[:NC]), ("core",))
    nin = n_params + len(out_names)
    fn = jax.jit(
        shard_map(_body, mesh=mesh,
                  in_specs=(PartitionSpec("core"),) * nin,
                  out_specs=(PartitionSpec("core"),) * len(out_names),
                  check_rep=False),
        donate_argnums=tuple(range(n_params, nin)),
    )
    _STATE["fn"] = fn
    _STATE["in_names"] = in_names
    from jax.sharding import NamedSharding
    shard = NamedSharding(mesh, PartitionSpec("core"))
    _STATE["shard"] = shard
    _STATE["put"] = lambda a: jax.device_put(a, shard)

    # Warm the whole path (XLA + NEFF compile + device load) with dummy data.
    f16 = np.float16
    dummy = _dummy_inputs()
    args = [dummy[n] for n in in_names] + [np.zeros((NC * NG, D), np.float32)]
    (out,) = fn(*args)
    out.block_until_ready()

    # constants stay resident on device across calls
    _STATE["iota"] = _STATE["put"](
        np.tile(np.arange(128, dtype=f16), (NC * 128, 1)))
    _STATE["ident"] = _STATE["put"](np.tile(np.eye(128, dtype=f16), (NC, 1)))


def _dummy_inputs():
    f16 = np.float16
    return {
        "x": np.zeros((NPAD, D), f16),
        "idx": np.zeros((NC * 128, NT), np.int32),
        "dstl": np.zeros((NC * 128, NT), f16),
        "stv": np.zeros((NC * 5, PER), f16),
        "batchv": np.zeros((NC * 128, NW), f16),
        "iota": np.zeros((NC * 128, 128), f16),
        "ident": np.zeros((NC * 128, 128), f16),
        **{f"w{li}": np.zeros((NC * D, D), f16) for li in range(3)},
        **{f"wb{li}": np.zeros((NC * 5, D), f16) for li in range(3)},
    }


def kernel(**inputs):
    _ensure_ready()
    f16 = np.float16
    put = _STATE["put"]

    x = np.asarray(inputs["x"], dtype=np.float32)
    ei = np.asarray(inputs["edge_index"])
    ea = np.asarray(inputs["edge_attr"], dtype=np.float32)
    batch = np.asarray(inputs["batch"]).astype(np.int64)
    src, dst = ei[0].astype(np.int64), ei[1].astype(np.int64)

    glob = {"iota": _STATE["iota"], "ident": _STATE["ident"]}

    # start the big x upload first; it overlaps the host-side edge prep
    xpad = np.zeros((NPAD, D), f16)
    xpad[:N] = x.astype(f16)
    glob["x"] = put(xpad)

    # S = segsum(edge_attr by dst) with a ones row folding in the bias
    stv = np.zeros((5, NPAD), f16)
    for k in range(ED):
        stv[k] = np.bincount(dst, weights=ea[:, k].astype(np.float64),
                             minlength=NPAD).astype(f16)
    stv[4, :N] = 1.0
    glob["stv"] = put(np.ascontiguousarray(
        stv.reshape(5, NC, PER).transpose(1, 0, 2)).reshape(NC * 5, PER))

    bpad = np.full(NPAD, 127, np.int64)
    bpad[:N] = batch
    glob["batchv"] = put(np.ascontiguousarray(
        bpad.reshape(NC, NW, 128).transpose(0, 2, 1)
    ).astype(f16).reshape(NC * 128, NW))

    for li, (Wn, bn) in enumerate((("W0", "b0"), ("W1", "b1"), ("W2", "b2"))):
        W = np.asarray(inputs[Wn], dtype=np.float32)
        b = np.asarray(inputs[bn], dtype=np.float32)
        glob[f"w{li}"] = put(np.tile(W[:D].astype(f16), (NC, 1)))
        glob[f"wb{li}"] = put(np.tile(
            np.concatenate([W[D:], b[None, :]], axis=0).astype(f16), (NC, 1)))

    # bucket edges by destination window, pad windows to K slots
    win0 = (dst >> 7).astype(np.int16)
    order = np.argsort(win0, kind="stable")
    dst_s = dst[order]
    src_s = src[order]
    win = win0[order].astype(np.int64)
    counts = np.bincount(win, minlength=NC * NW)
    assert counts.max() <= K, f"window overflow: {counts.max()} > {K}"
    starts = np.zeros(NC * NW + 1, np.int64)
    np.cumsum(counts, out=starts[1:])
    slot = np.arange(E) - starts[win]
    src_pad = np.full((NC * NW, K), NPAD - 1, np.int32)
    dstl_pad = np.zeros((NC * NW, K), f16)
    src_pad[win, slot] = src_s.astype(np.int32)
    dstl_pad[win, slot] = (dst_s & 127).astype(f16)
    glob["idx"] = put(np.ascontiguousarray(
        src_pad.reshape(NC, NW, KT, 128).transpose(0, 3, 1, 2)
    ).reshape(NC * 128, NT))
    glob["dstl"] = put(np.ascontiguousarray(
        dstl_pad.reshape(NC, NW, KT, 128).transpose(0, 3, 1, 2)
    ).reshape(NC * 128, NT))

    fn = _STATE["fn"]
    args = [glob[n] for n in _STATE["in_names"]] \
        + [np.zeros((NC * NG, D), np.float32)]
    (out,) = fn(*args)
    pooled_parts = np.asarray(out).reshape(NC, NG, D)
    pooled_sum = pooled_parts.sum(axis=0)

    counts_g = np.bincount(batch, minlength=NG).astype(np.float32)
    pooled = pooled_sum / np.maximum(counts_g, 1.0)[:, None]
    logits = pooled @ np.asarray(inputs["Wout"], np.float32) \
        + np.asarray(inputs["bout"], np.float32)
    mx = logits.max(axis=1, keepdims=True)
    lse = np.log(np.exp(logits - mx).sum(axis=1, keepdims=True)) + mx
    return (logits - lse).astype(np.float32)


try:
    _ensure_ready()
except Exception:
    _STATE.clear()
